# revision 1
# baseline (speedup 1.0000x reference)
"""AdaptiveAttentionLSTMCell fused kernel for one TRN2 chip (8 NeuronCores).

Math note: the reference applies softmax over a size-1 axis (zt is [B, K+1, 1],
softmax(axis=-1)), which is identically 1.0 for finite inputs. Hence
ct = sum_k v_expand[:, k, :] = v_seq.sum(axis=1) + st exactly, and the
W_z / U_z / W_h attention projections never affect the output. The kernel
therefore computes:

    z  = h_tm @ W_gates + inputs @ U_gates + b_gates          [B, 5U]
    ft,it,ot,gt = sigmoid(f,i,o,g);  at = tanh(a)
    mt = m_tm * ft + it * at
    tm = tanh(mt); ht = ot * tm; st = gt * tm
    out = (ht + st + v_seq.sum(1), ht, mt)

Distribution: 2-way data-parallel over batch x 4-way parallel over the unit
dim (each core owns all 5 gate blocks for its 256 units, so the gate
elementwise math stays local and no collective is needed). Host reassembles
the 8 output shards.

Matmul runs in bf16 (f32 PSUM accumulation); v_seq is reduced from bf16.
Relative error vs the f32 reference is ~2e-3.
"""

import numpy as np
import ml_dtypes

# Problem shape (hardcoded per the harness contract).
B, D_IN, UNITS, KF = 2048, 2048, 1024, 49
N_CORES = 8
PB, PU = 2, 4                 # batch shards x unit shards
B_L = B // PB                 # 1024 batch rows per core
U_L = UNITS // PU             # 256 units per core
K = UNITS + D_IN              # 3072 contraction dim
N_L = 5 * U_L                 # 1280 gate columns per core
P = 128                       # partitions
NB_T = B_L // P               # 8 batch tiles
NK_T = K // P                 # 24 k tiles
BF16 = ml_dtypes.bfloat16

_NC_CACHE = {}


def _build_nc(with_bias):
    import concourse.bacc as bacc
    import concourse.mybir as mybir
    import concourse.tile as tile

    dt = mybir.dt
    nc = bacc.Bacc("TRN2", target_bir_lowering=False, debug=False)

    aT = nc.dram_tensor("aT", [K, B_L], dt.bfloat16, kind="ExternalInput").ap()
    W = nc.dram_tensor("W", [K, N_L], dt.bfloat16, kind="ExternalInput").ap()
    m = nc.dram_tensor("m", [B_L, U_L], dt.float32, kind="ExternalInput").ap()
    v = nc.dram_tensor("v", [B_L, U_L, KF], dt.bfloat16, kind="ExternalInput").ap()
    if with_bias:
        bb = nc.dram_tensor("bb", [P, N_L], dt.float32, kind="ExternalInput").ap()
    o0 = nc.dram_tensor("o0", [B_L, U_L], dt.float32, kind="ExternalOutput").ap()
    o1 = nc.dram_tensor("o1", [B_L, U_L], dt.float32, kind="ExternalOutput").ap()
    o2 = nc.dram_tensor("o2", [B_L, U_L], dt.float32, kind="ExternalOutput").ap()

    NCH = ((0, 512), (512, 512), (1024, 256))  # psum-bank-aligned matmul chunks

    with tile.TileContext(nc) as tc:
        with (
            tc.tile_pool(name="resident", bufs=1) as rp,
            tc.tile_pool(name="vload", bufs=2) as vp,
            tc.tile_pool(name="work", bufs=2) as wp,
            tc.tile_pool(name="psum", bufs=2, space="PSUM") as pp,
        ):
            aT_sb = rp.tile([P, NK_T * B_L], dt.bfloat16)
            W_sb = rp.tile([P, NK_T * N_L], dt.bfloat16)
            for k in range(NK_T):
                nc.sync.dma_start(W_sb[:, k * N_L:(k + 1) * N_L], W[k * P:(k + 1) * P, :])
                nc.sync.dma_start(aT_sb[:, k * B_L:(k + 1) * B_L], aT[k * P:(k + 1) * P, :])
            if with_bias:
                bb_sb = rp.tile([P, N_L], dt.float32)
                nc.sync.dma_start(bb_sb[:], bb[:])

            for bt in range(NB_T):
                bs = slice(bt * P, (bt + 1) * P)
                z = pp.tile([P, N_L], dt.float32)
                for k in range(NK_T):
                    lhsT = aT_sb[:, k * B_L + bt * P: k * B_L + (bt + 1) * P]
                    for (n0, nw) in NCH:
                        nc.tensor.matmul(
                            z[:, n0:n0 + nw],
                            lhsT,
                            W_sb[:, k * N_L + n0: k * N_L + n0 + nw],
                            start=(k == 0),
                            stop=(k == NK_T - 1),
                        )
                if with_bias:
                    nc.vector.tensor_add(z[:], z[:], bb_sb[:])

                vt = vp.tile([P, U_L, KF], dt.bfloat16)
                nc.sync.dma_start(vt[:], v[bs, :, :])
                vs = wp.tile([P, U_L], dt.float32)
                nc.vector.tensor_reduce(vs[:], vt[:], axis=mybir.AxisListType.X,
                                        op=mybir.AluOpType.add)

                m_sb = wp.tile([P, U_L], dt.float32)
                nc.sync.dma_start(m_sb[:], m[bs, :])

                ft = wp.tile([P, U_L], dt.float32)
                it = wp.tile([P, U_L], dt.float32)
                ot = wp.tile([P, U_L], dt.float32)
                gt = wp.tile([P, U_L], dt.float32)
                at = wp.tile([P, U_L], dt.float32)
                Sig = mybir.ActivationFunctionType.Sigmoid
                Tanh = mybir.ActivationFunctionType.Tanh
                nc.scalar.activation(ft[:], z[:, 0 * U_L:1 * U_L], Sig)
                nc.scalar.activation(it[:], z[:, 1 * U_L:2 * U_L], Sig)
                nc.scalar.activation(ot[:], z[:, 2 * U_L:3 * U_L], Sig)
                nc.scalar.activation(gt[:], z[:, 3 * U_L:4 * U_L], Sig)
                nc.scalar.activation(at[:], z[:, 4 * U_L:5 * U_L], Tanh)

                t0 = wp.tile([P, U_L], dt.float32)
                t1 = wp.tile([P, U_L], dt.float32)
                mt = wp.tile([P, U_L], dt.float32)
                nc.vector.tensor_mul(t0[:], m_sb[:], ft[:])
                nc.vector.tensor_mul(t1[:], it[:], at[:])
                nc.vector.tensor_add(mt[:], t0[:], t1[:])

                tm = wp.tile([P, U_L], dt.float32)
                nc.scalar.activation(tm[:], mt[:], Tanh)

                ht = wp.tile([P, U_L], dt.float32)
                og = wp.tile([P, U_L], dt.float32)
                hs = wp.tile([P, U_L], dt.float32)
                o0t = wp.tile([P, U_L], dt.float32)
                nc.vector.tensor_mul(ht[:], ot[:], tm[:])
                nc.vector.tensor_add(og[:], ot[:], gt[:])
                nc.vector.tensor_mul(hs[:], og[:], tm[:])
                nc.vector.tensor_add(o0t[:], hs[:], vs[:])

                nc.sync.dma_start(o0[bs, :], o0t[:])
                nc.sync.dma_start(o1[bs, :], ht[:])
                nc.sync.dma_start(o2[bs, :], mt[:])

    nc.compile()
    return nc


def _get_nc(with_bias):
    key = bool(with_bias)
    if key not in _NC_CACHE:
        _NC_CACHE[key] = _build_nc(key)
    return _NC_CACHE[key]


def _prepare_in_maps(inputs):
    x = np.asarray(inputs["inputs"], np.float32)
    h = np.asarray(inputs["h_tm"], np.float32)
    m = np.asarray(inputs["m_tm"], np.float32)
    v = np.asarray(inputs["v_seq"], np.float32)
    Wg = np.asarray(inputs["W_gates"], np.float32)
    Ug = np.asarray(inputs["U_gates"], np.float32)
    bg = np.asarray(inputs["b_gates"], np.float32)

    with_bias = bool(np.any(bg))
    A_T = np.ascontiguousarray(np.concatenate([h, x], axis=1).T.astype(BF16))  # [K, B]
    W_full = np.concatenate([Wg, Ug], axis=0)                                   # [K, 5U]

    in_maps = []
    for c in range(N_CORES):
        pb, pu = divmod(c, PU)
        bsl = slice(pb * B_L, (pb + 1) * B_L)
        cols = np.concatenate(
            [np.arange(j * UNITS + pu * U_L, j * UNITS + (pu + 1) * U_L)
             for j in range(5)])
        im = {
            "aT": np.ascontiguousarray(A_T[:, bsl]),
            "W": np.ascontiguousarray(W_full[:, cols].astype(BF16)),
            "m": np.ascontiguousarray(m[bsl, pu * U_L:(pu + 1) * U_L]),
            "v": np.ascontiguousarray(
                v[bsl, :, pu * U_L:(pu + 1) * U_L].transpose(0, 2, 1).astype(BF16)),
        }
        if with_bias:
            im["bb"] = np.ascontiguousarray(
                np.broadcast_to(bg[cols], (P, N_L)).astype(np.float32))
        in_maps.append(im)
    return in_maps, with_bias


def _assemble(results):
    outs = []
    for name in ("o0", "o1", "o2"):
        full = np.empty((B, UNITS), np.float32)
        for c in range(N_CORES):
            pb, pu = divmod(c, PU)
            full[pb * B_L:(pb + 1) * B_L, pu * U_L:(pu + 1) * U_L] = results[c][name]
        outs.append(full)
    return tuple(outs)


def _run(inputs, **spmd_kwargs):
    from concourse.bass_utils import run_bass_kernel_spmd

    in_maps, with_bias = _prepare_in_maps(inputs)
    nc = _get_nc(with_bias)
    res = run_bass_kernel_spmd(nc, in_maps, core_ids=list(range(N_CORES)),
                               **spmd_kwargs)
    return _assemble(res.results), res


def kernel(**inputs):
    outs, _ = _run(inputs)
    return outs


# revision 10
# speedup vs baseline: 1.1026x; 1.1026x over previous
"""AdaptiveAttentionLSTMCell fused kernel for one TRN2 chip (8 NeuronCores).

Math note: the reference applies softmax over a size-1 axis (zt is [B, K+1, 1],
softmax(axis=-1)), which is identically 1.0 for finite inputs. Hence
ct = sum_k v_expand[:, k, :] = v_seq.sum(axis=1) + st exactly, and the
W_z / U_z / W_h attention projections never affect the output. The kernel
therefore computes:

    z  = h_tm @ W_gates + inputs @ U_gates + b_gates          [B, 5U]
    ft,it,ot,gt = sigmoid(f,i,o,g);  at = tanh(a)
    mt = m_tm * ft + it * at
    tm = tanh(mt); ht = ot * tm; st = gt * tm
    out = (ht + st + v_seq.sum(1), ht, mt)

Distribution: 2-way data-parallel over batch x 4-way parallel over the unit
dim (each core owns all 5 gate blocks for its 256 units, so the gate
elementwise math stays local and no collective is needed). Host reassembles
the 8 output shards.

Per-core schedule: the [1024, 3072] @ [3072, 1280] gate matmul runs as 6
phases (2 batch groups x 3 psum-bank-aligned column chunks); gate columns
are host-reordered to [f, i | a, o | g] so each phase's outputs feed the
gate nonlinearities as soon as the phase retires. v_seq is pre-folded by an
accumulating DMA (49 -> 24 slices) and reduced by an in-place bf16 add tree
on the vector engine. Matmuls run in bf16 with f32 PSUM accumulation.
"""

import numpy as np
import ml_dtypes

# Problem shape (hardcoded per the harness contract).
B, D_IN, UNITS, KF = 2048, 2048, 1024, 49
N_CORES = 8
PB, PU = 2, 4                 # batch shards x unit shards
B_L = B // PB                 # 1024 batch rows per core
U_L = UNITS // PU             # 256 units per core
K = UNITS + D_IN              # 3072 contraction dim
N_L = 5 * U_L                 # 1280 gate columns per core
P = 128                       # partitions
NB_T = B_L // P               # 8 batch tiles
NK_T = K // P                 # 24 k tiles
GRP = 4                       # batch tiles per phase group
BF16 = ml_dtypes.bfloat16

_NC_CACHE = {}


def _build_nc(with_bias):
    import concourse.bacc as bacc
    import concourse.mybir as mybir
    import concourse.tile as tile

    dt = mybir.dt
    f32, bf = dt.float32, dt.bfloat16
    Sig = mybir.ActivationFunctionType.Sigmoid
    Tanh = mybir.ActivationFunctionType.Tanh
    add = mybir.AluOpType.add
    nc = bacc.Bacc("TRN2", target_bir_lowering=False, debug=False)

    aT = nc.dram_tensor("aT", [K, B_L], bf, kind="ExternalInput").ap()
    W = nc.dram_tensor("W", [K, N_L], bf, kind="ExternalInput").ap()
    m = nc.dram_tensor("m", [B_L, U_L], bf, kind="ExternalInput").ap()
    v = nc.dram_tensor("v", [B_L, KF, U_L], bf, kind="ExternalInput").ap()
    if with_bias:
        bb = nc.dram_tensor("bb", [P, N_L], f32, kind="ExternalInput").ap()
    o0 = nc.dram_tensor("o0", [B_L, U_L], f32, kind="ExternalOutput").ap()
    o1 = nc.dram_tensor("o1", [B_L, U_L], f32, kind="ExternalOutput").ap()
    o2 = nc.dram_tensor("o2", [B_L, U_L], f32, kind="ExternalOutput").ap()
    import os
    DEBUG = bool(os.environ.get("KM_DEBUG"))
    if DEBUG:
        dgt = nc.dram_tensor("dgt", [B_L, U_L], f32, kind="ExternalOutput").ap()
        dvs = nc.dram_tensor("dvs", [B_L, U_L], f32, kind="ExternalOutput").ap()

    # column chunks within the reordered [f i a o g] gate layout
    PH = ((0, 512), (512, 512), (1024, 256))

    with tile.TileContext(nc) as tc:
        with (
            tc.tile_pool(name="resident", bufs=1) as rp,
            tc.tile_pool(name="vload", bufs=2) as vp,
            tc.tile_pool(name="g8", bufs=5) as g8,     # tiles live across a phase
            tc.tile_pool(name="vs8", bufs=4) as vs8,   # v sums
            tc.tile_pool(name="work", bufs=2) as wp,   # short-lived epilogue tiles
            tc.tile_pool(name="psum", bufs=8, space="PSUM") as pp,
        ):
            aT_sb = rp.tile([P, NK_T * B_L], bf)
            W_sb = rp.tile([P, NK_T * N_L], bf)
            for k in range(NK_T):
                nc.sync.dma_start(W_sb[:, k * N_L:(k + 1) * N_L], W[k * P:(k + 1) * P, :])
                nc.sync.dma_start(aT_sb[:, k * B_L:(k + 1) * B_L], aT[k * P:(k + 1) * P, :])
            if with_bias:
                bb_sb = rp.tile([P, N_L], f32)
                nc.sync.dma_start(bb_sb[:], bb[:])

            # per-bt state carried across phases
            S = {}

            def mm_phase(group, ph):
                n0, nw = PH[ph]
                zs = {}
                for bt in group:
                    zs[bt] = pp.tile([P, 512], f32, tag="z", name=f"z_{ph}_{bt}")
                for k in range(NK_T):
                    for bt in group:
                        lhsT = aT_sb[:, k * B_L + bt * P: k * B_L + (bt + 1) * P]
                        nc.tensor.matmul(
                            zs[bt][:, :nw],
                            lhsT,
                            W_sb[:, k * N_L + n0: k * N_L + n0 + nw],
                            start=(k == 0),
                            stop=(k == NK_T - 1),
                        )
                if with_bias:
                    for bt in group:
                        nc.vector.tensor_add(zs[bt][:, :nw], zs[bt][:, :nw],
                                             bb_sb[:, n0:n0 + nw])
                return zs

            def v_sum(bt):
                bs = slice(bt * P, (bt + 1) * P)
                vt = vp.tile([P, KF, U_L], bf)
                nc.gpsimd.dma_start(vt[:], v[bs, :, :])
                nc.vector.tensor_add(vt[:, 1:25, :], vt[:, 1:25, :], vt[:, 25:49, :])
                # in-place bf16 tree over the 25 folded slices: 1 + 24
                nc.vector.tensor_add(vt[:, 1:13, :], vt[:, 1:13, :], vt[:, 13:25, :])
                nc.vector.tensor_add(vt[:, 1:7, :], vt[:, 1:7, :], vt[:, 7:13, :])
                nc.vector.tensor_add(vt[:, 1:4, :], vt[:, 1:4, :], vt[:, 4:7, :])
                nc.vector.tensor_add(vt[:, 1, :], vt[:, 1, :], vt[:, 2, :])
                nc.vector.tensor_add(vt[:, 1, :], vt[:, 1, :], vt[:, 3, :])
                vs = vs8.tile([P, U_L], f32)
                nc.vector.tensor_add(vs[:], vt[:, 0, :], vt[:, 1, :])
                return vs

            for group in (range(0, GRP), range(GRP, NB_T)):
                z0 = mm_phase(group, 0)            # f, i
                for bt in group:
                    ft = g8.tile([P, U_L], bf, tag="ft")
                    it = g8.tile([P, U_L], bf, tag="it")
                    nc.scalar.activation(ft[:], z0[bt][:, 0:U_L], Sig)
                    nc.scalar.activation(it[:], z0[bt][:, U_L:2 * U_L], Sig)
                    S[bt] = {"ft": ft, "it": it}

                z1 = mm_phase(group, 1)            # a, o
                for bt in group:
                    st_ = S[bt]
                    at = wp.tile([P, U_L], bf, tag="at")
                    ot = g8.tile([P, U_L], bf, tag="ot")
                    nc.scalar.activation(at[:], z1[bt][:, 0:U_L], Tanh)
                    nc.scalar.activation(ot[:], z1[bt][:, U_L:2 * U_L], Sig)
                    m_sb = wp.tile([P, U_L], bf, tag="m")
                    nc.sync.dma_start(m_sb[:], m[bt * P:(bt + 1) * P, :])
                    t0 = wp.tile([P, U_L], bf, tag="t0")
                    t1 = wp.tile([P, U_L], bf, tag="t1")
                    mt = wp.tile([P, U_L], f32, tag="mt")
                    nc.vector.tensor_mul(t0[:], m_sb[:], st_["ft"][:])
                    nc.vector.tensor_mul(t1[:], st_["it"][:], at[:])
                    nc.vector.tensor_add(mt[:], t0[:], t1[:])
                    tm = g8.tile([P, U_L], bf, tag="tm")
                    nc.scalar.activation(tm[:], mt[:], Tanh)
                    ht = g8.tile([P, U_L], f32, tag="ht")
                    nc.vector.tensor_mul(ht[:], ot[:], tm[:])
                    nc.sync.dma_start(o1[bt * P:(bt + 1) * P, :], ht[:])
                    nc.sync.dma_start(o2[bt * P:(bt + 1) * P, :], mt[:])
                    st_.update(ot=ot, tm=tm)

                z2 = mm_phase(group, 2)            # g
                for bt in group:
                    st_ = S[bt]
                    vs = v_sum(bt)
                    gt = wp.tile([P, U_L], bf, tag="gt")
                    nc.scalar.activation(gt[:], z2[bt][:, 0:U_L], Sig)
                    og = wp.tile([P, U_L], bf, tag="og")
                    hs = wp.tile([P, U_L], f32, tag="hs")
                    o0t = wp.tile([P, U_L], f32, tag="o0t")
                    # (ot + gt) * tm == ht + st, so out0 = hs + vsum
                    nc.vector.tensor_add(og[:], st_["ot"][:], gt[:])
                    nc.vector.tensor_mul(hs[:], og[:], st_["tm"][:])
                    nc.vector.tensor_add(o0t[:], hs[:], vs[:])
                    nc.sync.dma_start(o0[bt * P:(bt + 1) * P, :], o0t[:])
                    if DEBUG:
                        dg = wp.tile([P, U_L], f32, tag="dg")
                        nc.vector.tensor_copy(dg[:], gt[:])
                        nc.sync.dma_start(dgt[bt * P:(bt + 1) * P, :], dg[:])
                        nc.sync.dma_start(dvs[bt * P:(bt + 1) * P, :], vs[:])
                    del S[bt]

    nc.compile()
    return nc


def _get_nc(with_bias):
    key = bool(with_bias)
    if key not in _NC_CACHE:
        _NC_CACHE[key] = _build_nc(key)
    return _NC_CACHE[key]


def _prepare_in_maps(inputs):
    x = np.asarray(inputs["inputs"], np.float32)
    h = np.asarray(inputs["h_tm"], np.float32)
    m = np.asarray(inputs["m_tm"], np.float32)
    v = np.asarray(inputs["v_seq"], np.float32)
    Wg = np.asarray(inputs["W_gates"], np.float32)
    Ug = np.asarray(inputs["U_gates"], np.float32)
    bg = np.asarray(inputs["b_gates"], np.float32)

    with_bias = bool(np.any(bg))
    A_T = np.ascontiguousarray(np.concatenate([h, x], axis=1).T.astype(BF16))  # [K, B]
    W_full = np.concatenate([Wg, Ug], axis=0)                                   # [K, 5U]

    in_maps = []
    for c in range(N_CORES):
        pb, pu = divmod(c, PU)
        bsl = slice(pb * B_L, (pb + 1) * B_L)
        u = np.arange(pu * U_L, (pu + 1) * U_L)
        # gate-block order [f, i, a, o, g] (reference stacks [f, i, o, g, a])
        cols = np.concatenate([j * UNITS + u for j in (0, 1, 4, 2, 3)])
        im = {
            "aT": np.ascontiguousarray(A_T[:, bsl]),
            "W": np.ascontiguousarray(W_full[:, cols].astype(BF16)),
            "m": np.ascontiguousarray(m[bsl, pu * U_L:(pu + 1) * U_L].astype(BF16)),
            "v": np.ascontiguousarray(v[bsl, :, pu * U_L:(pu + 1) * U_L].astype(BF16)),
        }
        if with_bias:
            im["bb"] = np.ascontiguousarray(
                np.broadcast_to(bg[cols], (P, N_L)).astype(np.float32))
        in_maps.append(im)
    return in_maps, with_bias


def _assemble(results):
    outs = []
    for name in ("o0", "o1", "o2"):
        full = np.empty((B, UNITS), np.float32)
        for c in range(N_CORES):
            pb, pu = divmod(c, PU)
            full[pb * B_L:(pb + 1) * B_L, pu * U_L:(pu + 1) * U_L] = results[c][name]
        outs.append(full)
    return tuple(outs)


def _run(inputs, **spmd_kwargs):
    from concourse.bass_utils import run_bass_kernel_spmd

    in_maps, with_bias = _prepare_in_maps(inputs)
    nc = _get_nc(with_bias)
    res = run_bass_kernel_spmd(nc, in_maps, core_ids=list(range(N_CORES)),
                               **spmd_kwargs)
    return _assemble(res.results), res


def kernel(**inputs):
    outs, _ = _run(inputs)
    return outs


# revision 11
# speedup vs baseline: 1.3390x; 1.2144x over previous
"""AdaptiveAttentionLSTMCell fused kernel for one TRN2 chip (8 NeuronCores).

Math note: the reference applies softmax over a size-1 axis (zt is [B, K+1, 1],
softmax(axis=-1)), which is identically 1.0 for finite inputs. Hence
ct = sum_k v_expand[:, k, :] = v_seq.sum(axis=1) + st exactly, and the
W_z / U_z / W_h attention projections never affect the output. The kernel
therefore computes:

    z  = h_tm @ W_gates + inputs @ U_gates + b_gates          [B, 5U]
    ft,it,ot,gt = sigmoid(f,i,o,g);  at = tanh(a)
    mt = m_tm * ft + it * at
    tm = tanh(mt); ht = ot * tm; st = gt * tm
    out = (ht + st + v_seq.sum(1), ht, mt)

Distribution: 2-way data-parallel over batch x 4-way parallel over the unit
dim (each core owns all 5 gate blocks for its 256 units, so the gate
elementwise math stays local and no collective is needed). Host reassembles
the 8 output shards.

Per-core schedule: the [1024, 3072] @ [3072, 1280] gate matmul runs as 6
phases (2 batch groups x 3 psum-bank-aligned column chunks); gate columns
are host-reordered to [f, i | a, o | g] so each phase's outputs feed the
gate nonlinearities as soon as the phase retires. v_seq is pre-folded by an
accumulating DMA (49 -> 24 slices) and reduced by an in-place bf16 add tree
on the vector engine. Matmuls run in bf16 with f32 PSUM accumulation.
"""

import numpy as np
import ml_dtypes

# Problem shape (hardcoded per the harness contract).
B, D_IN, UNITS, KF = 2048, 2048, 1024, 49
N_CORES = 8
PB, PU = 2, 4                 # batch shards x unit shards
B_L = B // PB                 # 1024 batch rows per core
U_L = UNITS // PU             # 256 units per core
K = UNITS + D_IN              # 3072 contraction dim
N_L = 5 * U_L                 # 1280 gate columns per core
P = 128                       # partitions
NB_T = B_L // P               # 8 batch tiles
NK_T = K // P                 # 24 k tiles
GRP = 4                       # batch tiles per phase group
BF16 = ml_dtypes.bfloat16

_NC_CACHE = {}


def _build_nc(with_bias):
    import concourse.bacc as bacc
    import concourse.mybir as mybir
    import concourse.tile as tile

    dt = mybir.dt
    f32, bf = dt.float32, dt.bfloat16
    Sig = mybir.ActivationFunctionType.Sigmoid
    Tanh = mybir.ActivationFunctionType.Tanh
    add = mybir.AluOpType.add
    nc = bacc.Bacc("TRN2", target_bir_lowering=False, debug=False)

    aT = nc.dram_tensor("aT", [K, B_L], bf, kind="ExternalInput").ap()
    W = nc.dram_tensor("W", [K, N_L], bf, kind="ExternalInput").ap()
    m = nc.dram_tensor("m", [B_L, U_L], bf, kind="ExternalInput").ap()
    v = nc.dram_tensor("v", [B_L, KF, U_L], bf, kind="ExternalInput").ap()
    if with_bias:
        bb = nc.dram_tensor("bb", [P, N_L], f32, kind="ExternalInput").ap()
    o0 = nc.dram_tensor("o0", [B_L, U_L], f32, kind="ExternalOutput").ap()
    o1 = nc.dram_tensor("o1", [B_L, U_L], f32, kind="ExternalOutput").ap()
    o2 = nc.dram_tensor("o2", [B_L, U_L], f32, kind="ExternalOutput").ap()
    import os
    DEBUG = bool(os.environ.get("KM_DEBUG"))
    if DEBUG:
        dgt = nc.dram_tensor("dgt", [B_L, U_L], f32, kind="ExternalOutput").ap()
        dvs = nc.dram_tensor("dvs", [B_L, U_L], f32, kind="ExternalOutput").ap()

    # column chunks within the reordered [f i a o g] gate layout
    PH = ((0, 512), (512, 512), (1024, 256))

    with tile.TileContext(nc) as tc:
        with (
            tc.tile_pool(name="resident", bufs=1) as rp,
            tc.tile_pool(name="vload", bufs=2) as vp,
            tc.tile_pool(name="g8", bufs=5) as g8,     # tiles live across a phase
            tc.tile_pool(name="vs8", bufs=4) as vs8,   # v sums
            tc.tile_pool(name="work", bufs=2) as wp,   # short-lived epilogue tiles
            tc.tile_pool(name="psum", bufs=8, space="PSUM") as pp,
        ):
            aT_sb = rp.tile([P, NK_T * B_L], bf)
            W_sb = rp.tile([P, NK_T * N_L], bf)
            for k in range(NK_T):
                nc.sync.dma_start(W_sb[:, k * N_L:(k + 1) * N_L], W[k * P:(k + 1) * P, :])
                nc.sync.dma_start(aT_sb[:, k * B_L:(k + 1) * B_L], aT[k * P:(k + 1) * P, :])
            if with_bias:
                bb_sb = rp.tile([P, N_L], f32)
                nc.sync.dma_start(bb_sb[:], bb[:])

            # per-bt state carried across phases
            S = {}
            m_sbs = {}
            for bt in range(NB_T):
                m_sb = g8.tile([P, U_L], bf, tag=f"m{bt}", name=f"m_sb{bt}", bufs=1)
                nc.sync.dma_start(m_sb[:], m[bt * P:(bt + 1) * P, :])
                m_sbs[bt] = m_sb

            def mm_phase(group, ph):
                n0, nw = PH[ph]
                zs = {}
                for bt in group:
                    zs[bt] = pp.tile([P, 512], f32, tag="z", name=f"z_{ph}_{bt}")
                for k in range(NK_T):
                    for bt in group:
                        lhsT = aT_sb[:, k * B_L + bt * P: k * B_L + (bt + 1) * P]
                        nc.tensor.matmul(
                            zs[bt][:, :nw],
                            lhsT,
                            W_sb[:, k * N_L + n0: k * N_L + n0 + nw],
                            start=(k == 0),
                            stop=(k == NK_T - 1),
                        )
                if with_bias:
                    for bt in group:
                        nc.vector.tensor_add(zs[bt][:, :nw], zs[bt][:, :nw],
                                             bb_sb[:, n0:n0 + nw])
                return zs

            def v_load(bt):
                bs = slice(bt * P, (bt + 1) * P)
                vt = vp.tile([P, KF, U_L], bf)
                nc.sync.dma_start(vt[:], v[bs, :, :])
                return vt

            def v_sum(bt, vt):
                nc.vector.tensor_add(vt[:, 1:25, :], vt[:, 1:25, :], vt[:, 25:49, :])
                # in-place bf16 tree over the 25 folded slices: 1 + 24
                nc.vector.tensor_add(vt[:, 1:13, :], vt[:, 1:13, :], vt[:, 13:25, :])
                nc.vector.tensor_add(vt[:, 1:7, :], vt[:, 1:7, :], vt[:, 7:13, :])
                nc.vector.tensor_add(vt[:, 1:4, :], vt[:, 1:4, :], vt[:, 4:7, :])
                nc.vector.tensor_add(vt[:, 1, :], vt[:, 1, :], vt[:, 2, :])
                nc.vector.tensor_add(vt[:, 1, :], vt[:, 1, :], vt[:, 3, :])
                vs = vs8.tile([P, U_L], f32)
                nc.vector.tensor_add(vs[:], vt[:, 0, :], vt[:, 1, :])
                return vs

            for group in (range(0, GRP), range(GRP, NB_T)):
                vts = {bt: v_load(bt) for bt in group}
                z0 = mm_phase(group, 0)            # f, i
                for bt in group:
                    ft = g8.tile([P, U_L], bf, tag="ft")
                    it = g8.tile([P, U_L], bf, tag="it")
                    nc.scalar.activation(ft[:], z0[bt][:, 0:U_L], Sig)
                    nc.scalar.activation(it[:], z0[bt][:, U_L:2 * U_L], Sig)
                    S[bt] = {"ft": ft, "it": it}

                z1 = mm_phase(group, 1)            # a, o
                for bt in group:
                    st_ = S[bt]
                    at = wp.tile([P, U_L], bf, tag="at")
                    ot = g8.tile([P, U_L], bf, tag="ot")
                    nc.scalar.activation(at[:], z1[bt][:, 0:U_L], Tanh)
                    nc.scalar.activation(ot[:], z1[bt][:, U_L:2 * U_L], Sig)
                    m_sb = m_sbs[bt]
                    t0 = wp.tile([P, U_L], bf, tag="t0")
                    t1 = wp.tile([P, U_L], bf, tag="t1")
                    mt = wp.tile([P, U_L], f32, tag="mt")
                    nc.vector.tensor_mul(t0[:], m_sb[:], st_["ft"][:])
                    nc.vector.tensor_mul(t1[:], st_["it"][:], at[:])
                    nc.vector.tensor_add(mt[:], t0[:], t1[:])
                    tm = g8.tile([P, U_L], bf, tag="tm")
                    nc.scalar.activation(tm[:], mt[:], Tanh)
                    ht = g8.tile([P, U_L], f32, tag="ht")
                    nc.vector.tensor_mul(ht[:], ot[:], tm[:])
                    nc.sync.dma_start(o1[bt * P:(bt + 1) * P, :], ht[:])
                    nc.sync.dma_start(o2[bt * P:(bt + 1) * P, :], mt[:])
                    st_.update(ot=ot, tm=tm)

                z2 = mm_phase(group, 2)            # g
                for bt in group:
                    st_ = S[bt]
                    vs = v_sum(bt, vts[bt])
                    gt = wp.tile([P, U_L], bf, tag="gt")
                    nc.scalar.activation(gt[:], z2[bt][:, 0:U_L], Sig)
                    og = wp.tile([P, U_L], bf, tag="og")
                    hs = wp.tile([P, U_L], f32, tag="hs")
                    o0t = wp.tile([P, U_L], f32, tag="o0t")
                    # (ot + gt) * tm == ht + st, so out0 = hs + vsum
                    nc.vector.tensor_add(og[:], st_["ot"][:], gt[:])
                    nc.vector.tensor_mul(hs[:], og[:], st_["tm"][:])
                    nc.vector.tensor_add(o0t[:], hs[:], vs[:])
                    nc.sync.dma_start(o0[bt * P:(bt + 1) * P, :], o0t[:])
                    if DEBUG:
                        dg = wp.tile([P, U_L], f32, tag="dg")
                        nc.vector.tensor_copy(dg[:], gt[:])
                        nc.sync.dma_start(dgt[bt * P:(bt + 1) * P, :], dg[:])
                        nc.sync.dma_start(dvs[bt * P:(bt + 1) * P, :], vs[:])
                    del S[bt]

    nc.compile()
    return nc


def _get_nc(with_bias):
    key = bool(with_bias)
    if key not in _NC_CACHE:
        _NC_CACHE[key] = _build_nc(key)
    return _NC_CACHE[key]


def _prepare_in_maps(inputs):
    x = np.asarray(inputs["inputs"], np.float32)
    h = np.asarray(inputs["h_tm"], np.float32)
    m = np.asarray(inputs["m_tm"], np.float32)
    v = np.asarray(inputs["v_seq"], np.float32)
    Wg = np.asarray(inputs["W_gates"], np.float32)
    Ug = np.asarray(inputs["U_gates"], np.float32)
    bg = np.asarray(inputs["b_gates"], np.float32)

    with_bias = bool(np.any(bg))
    A_T = np.ascontiguousarray(np.concatenate([h, x], axis=1).T.astype(BF16))  # [K, B]
    W_full = np.concatenate([Wg, Ug], axis=0)                                   # [K, 5U]

    in_maps = []
    for c in range(N_CORES):
        pb, pu = divmod(c, PU)
        bsl = slice(pb * B_L, (pb + 1) * B_L)
        u = np.arange(pu * U_L, (pu + 1) * U_L)
        # gate-block order [f, i, a, o, g] (reference stacks [f, i, o, g, a])
        cols = np.concatenate([j * UNITS + u for j in (0, 1, 4, 2, 3)])
        im = {
            "aT": np.ascontiguousarray(A_T[:, bsl]),
            "W": np.ascontiguousarray(W_full[:, cols].astype(BF16)),
            "m": np.ascontiguousarray(m[bsl, pu * U_L:(pu + 1) * U_L].astype(BF16)),
            "v": np.ascontiguousarray(v[bsl, :, pu * U_L:(pu + 1) * U_L].astype(BF16)),
        }
        if with_bias:
            im["bb"] = np.ascontiguousarray(
                np.broadcast_to(bg[cols], (P, N_L)).astype(np.float32))
        in_maps.append(im)
    return in_maps, with_bias


def _assemble(results):
    outs = []
    for name in ("o0", "o1", "o2"):
        full = np.empty((B, UNITS), np.float32)
        for c in range(N_CORES):
            pb, pu = divmod(c, PU)
            full[pb * B_L:(pb + 1) * B_L, pu * U_L:(pu + 1) * U_L] = results[c][name]
        outs.append(full)
    return tuple(outs)


def _run(inputs, **spmd_kwargs):
    from concourse.bass_utils import run_bass_kernel_spmd

    in_maps, with_bias = _prepare_in_maps(inputs)
    nc = _get_nc(with_bias)
    res = run_bass_kernel_spmd(nc, in_maps, core_ids=list(range(N_CORES)),
                               **spmd_kwargs)
    return _assemble(res.results), res


def kernel(**inputs):
    outs, _ = _run(inputs)
    return outs
